# revision 1
# baseline (speedup 1.0000x reference)
"""Trainium2 Bass kernel for the ConvNet problem.

Pipeline per 512-sample sub-tile (feature-major after a PE transpose):
  signal[.,50,3] -> running std (banded matmuls for windowed sums) -> conv1
  (dense 120->190 matmul) -> conv2 (dense 190->360) -> fc1 -> fc2 -> pamap
  (weight-streaming matmul producing sample-major logits) -> log_softmax.

Sharding: pure data parallelism, batch split 8 ways across NeuronCores.
"""

import math
import os

import numpy as np

# ---------------------------------------------------------------------------
# Problem constants (hardcoded; kernel.py must be self-contained)
# ---------------------------------------------------------------------------
B_TOTAL, L, C = 131072, 50, 3
WIN = 10
NCORES = 8
B_CORE = B_TOTAL // NCORES          # 16384
G_SUPER = 16                        # samples per partition per super-tile
SUPER = 128 * G_SUPER               # 2048 samples per super-tile
N_SUPER_FULL = B_CORE // SUPER      # 8
NSUB = 512                          # samples per sub-tile (matmul N)
SUBQ = SUPER // NSUB                # 4 sub-tiles per super-tile
F_IN = L * C                        # 150
F_USE = 147                         # features actually consumed (x[49,:] unused)
F_STD = 120                         # 40 windows x 3 channels
F_C1 = 190                          # 38 x 5
F_C2 = 360                          # 36 x 10
F_FC1 = 256
F_FC2 = 64
F_OUT = 12

# debug knob (harness never sets this; default = full problem)
_N_SUPER = int(os.environ.get("ATRN_NSUPER", str(N_SUPER_FULL)))


# ---------------------------------------------------------------------------
# Tile drain patch: walrus in this container rejects >2 sem waits on a
# CTRL-class (Drain) instruction.  Spread the end-of-kernel global-clock waits
# across per-proc SP nops (one sem each) before an unadorned drain.
# ---------------------------------------------------------------------------
def _install_drain_patch():
    import concourse.tile as tile
    from concourse.tile_scheduler import N_PROCS
    from concourse.vector_clock import ScopedClock, VectorClock

    if getattr(tile.TileContext, "_drain_patch_installed", False):
        return

    def _patched_drain_and_barrier(self, tick_clock, wait_clock):
        nc = self.nc
        gc = tick_clock.global_clock
        for p in range(N_PROCS):
            if gc[p] <= 0:
                continue
            v = [0] * N_PROCS
            v[p] = gc[p]
            nop = nc.sync.nop()
            wait_clock.add_sem_waits(nop.ins, ScopedClock({None: VectorClock(v)}))
        nc.sync.drain()
        nc.all_engine_barrier()
        assert self.sems is not None
        popped = nc._tile_sem_poison_stack.pop()
        assert popped is self._sem_poison
        nc.clear_and_free_semaphores(list(self.sems.allocated().values()))
        nc.all_engine_barrier()

    tile.TileContext._drain_and_barrier = _patched_drain_and_barrier
    tile.TileContext._drain_patch_installed = True


# ---------------------------------------------------------------------------
# Host-side weight preprocessing -> one [128, WF] f32 blob
# ---------------------------------------------------------------------------
class _BlobLayout:
    def __init__(self):
        self.cols = 0
        self.slots = {}

    def add(self, name, rows, cols):
        self.slots[name] = (self.cols, rows, cols)
        self.cols += cols
        return self.slots[name]


def _wait_cap(ins):
    # This walrus build rejects >1 sem wait on engine instructions.
    return 1


def _split_excess_waits(nc):
    """Hoist excess sem waits onto same-engine nops inserted just before."""
    from concourse import mybir

    ctr = 0
    for f in nc.m.functions:
        for blk in f.blocks:
            il = blk.instructions
            i = 0
            while i < len(il):
                ins = il[i]
                si = ins.sync_info
                cap = _wait_cap(ins)
                if si is not None and len(si.on_wait) > cap:
                    waits = list(si.on_wait)
                    extra, keep = waits[:-cap], waits[-cap:]
                    for w in extra:
                        ctr += 1
                        nop = mybir.InstNoOp(name=f"waitsplit-{ctr}",
                                             ins=[], outs=[])
                        nop.engine = ins.engine
                        nop.sync_info = type(si)(on_wait=[w], on_update=[])
                        nc.register_instruction(nop, overwrite=True)
                        il.insert(i, nop)
                        i += 1
                    ins.sync_info = type(si)(on_wait=keep,
                                             on_update=list(si.on_update))
                i += 1


_LAY = _BlobLayout()
_LAY.add("sA_a", 128, F_STD)    # windowed-sum (scaled) rows 0..127
_LAY.add("sA_b", 19, F_STD)     # rows 128..146
_LAY.add("s2_a", 128, F_STD)
_LAY.add("s2_b", 19, F_STD)
_LAY.add("c1_a", F_STD, 128)    # conv1 out chunk 0
_LAY.add("c1_b", F_STD, 62)     # conv1 out chunk 1
for m, mo in enumerate((128, 128, 104)):
    _LAY.add(f"c2a{m}", 128, mo)   # conv2, K rows 0..127
    _LAY.add(f"c2b{m}", 62, mo)    # conv2, K rows 128..189
for k, kc in enumerate((128, 128, 104)):
    for m in range(2):
        _LAY.add(f"f1_{k}_{m}", kc, 128)
for k in range(2):
    _LAY.add(f"f2_{k}", 128, F_FC2)
_LAY.add("wp", F_FC2, F_OUT)    # pamap rhs (weight-streaming)
_LAY.add("b1a", 128, 1)
_LAY.add("b1b", 62, 1)
for m, mo in enumerate((128, 128, 104)):
    _LAY.add(f"b2_{m}", mo, 1)
for m in range(2):
    _LAY.add(f"b3_{m}", 128, 1)
_LAY.add("b4", F_FC2, 1)
_LAY.add("bp", 128, F_OUT)      # pamap bias replicated across partitions
WF = _LAY.cols


def _build_blob(conv1_w, conv1_b, conv2_w, conv2_b, fc1_w, fc1_b, fc2_w, fc2_b,
                pamap_w, pamap_b):
    blob = np.zeros((128, WF), np.float32)

    def put(name, arr):
        off, rows, cols = _LAY.slots[name]
        assert arr.shape == (rows, cols), (name, arr.shape, (rows, cols))
        blob[:rows, off:off + cols] = arr

    # windowed sums: s'[3l+c] = (1/sqrt(10)) * sum_k x[3(l+k)+c]
    A = np.zeros((F_USE, F_STD), np.float32)
    for m in range(F_STD):
        l, c = divmod(m, 3)
        for k in range(WIN):
            A[3 * (l + k) + c, m] = 1.0
    put("sA_a", A[:128] / math.sqrt(10.0))
    put("sA_b", A[128:] / math.sqrt(10.0))
    put("s2_a", A[:128])
    put("s2_b", A[128:])

    # conv1 as dense [in 120, out 190]
    M1 = np.zeros((F_STD, F_C1), np.float32)
    for t in range(38):
        for o in range(5):
            for k in range(3):
                for i in range(3):
                    M1[3 * (t + k) + i, 5 * t + o] = conv1_w[o, i, k]
    put("c1_a", M1[:, :128])
    put("c1_b", M1[:, 128:])

    # conv2 as dense [in 190, out 360]
    M2 = np.zeros((F_C1, F_C2), np.float32)
    for t in range(36):
        for o in range(10):
            for k in range(3):
                for i in range(5):
                    M2[5 * (t + k) + i, 10 * t + o] = conv2_w[o, i, k]
    mo_off = (0, 128, 256, 360)
    for m in range(3):
        put(f"c2a{m}", M2[:128, mo_off[m]:mo_off[m + 1]])
        put(f"c2b{m}", M2[128:, mo_off[m]:mo_off[m + 1]])

    F1 = fc1_w.T.astype(np.float32)          # [360, 256]
    kc_off = (0, 128, 256, 360)
    for k in range(3):
        for m in range(2):
            put(f"f1_{k}_{m}", F1[kc_off[k]:kc_off[k + 1], m * 128:(m + 1) * 128])
    F2 = fc2_w.T.astype(np.float32)          # [256, 64]
    for k in range(2):
        put(f"f2_{k}", F2[k * 128:(k + 1) * 128])
    put("wp", pamap_w.T.astype(np.float32))  # [64, 12]

    b1 = np.zeros(F_C1, np.float32)
    for t in range(38):
        for o in range(5):
            b1[5 * t + o] = conv1_b[o]
    put("b1a", b1[:128, None])
    put("b1b", b1[128:, None])
    b2 = np.zeros(F_C2, np.float32)
    for t in range(36):
        for o in range(10):
            b2[10 * t + o] = conv2_b[o]
    for m in range(3):
        put(f"b2_{m}", b2[mo_off[m]:mo_off[m + 1], None])
    for m in range(2):
        put(f"b3_{m}", fc1_b[m * 128:(m + 1) * 128, None].astype(np.float32))
    put("b4", fc2_b[:, None].astype(np.float32))
    put("bp", np.tile(pamap_b.astype(np.float32)[None, :], (128, 1)))
    return blob


# ---------------------------------------------------------------------------
# Bass program
# ---------------------------------------------------------------------------
_PROGRAM = None


def _w(weights, name):
    off, rows, cols = _LAY.slots[name]
    return weights[0:rows, off:off + cols]


def _build_program(n_super):
    import concourse.bass as bass
    import concourse.tile as tile
    from concourse import mybir
    from concourse.masks import make_identity

    _install_drain_patch()
    f32 = mybir.dt.float32
    AF = mybir.ActivationFunctionType
    ALU = mybir.AluOpType

    b_core = n_super * SUPER
    nc = bass.Bass("TRN2", target_bir_lowering=False, debug=False,
                   num_devices=NCORES)
    sig = nc.dram_tensor("sig", [b_core, F_IN], f32, kind="ExternalInput")
    wb = nc.dram_tensor("wb", [128, WF], f32, kind="ExternalInput")
    out = nc.dram_tensor("out", [b_core, F_OUT], f32, kind="ExternalOutput")

    with tile.TileContext(nc) as tc:
        import contextlib
        with contextlib.ExitStack() as ctx:
            singles = ctx.enter_context(tc.tile_pool(name="singles", bufs=1))
            xin = ctx.enter_context(tc.tile_pool(name="xin", bufs=3))
            sbx = ctx.enter_context(tc.tile_pool(name="sbx", bufs=3))
            sbh = ctx.enter_context(tc.tile_pool(name="sbh", bufs=3))
            psA = ctx.enter_context(tc.tile_pool(name="psA", bufs=1, space="PSUM"))
            psB = psA
            outp = ctx.enter_context(tc.tile_pool(name="outp", bufs=2))

            weights = singles.tile([128, WF], f32)
            nc.sync.dma_start(out=weights, in_=wb[:, :])
            ident = singles.tile([128, 128], f32)
            make_identity(nc, ident)
            lgpool = ctx.enter_context(tc.tile_pool(name="lgp", bufs=n_super))
            logits_t = [lgpool.tile([128, G_SUPER * F_OUT], f32,
                                    name=f"lgt{i}", tag="lg")
                        for i in range(n_super)]

            sig_v = sig.rearrange("(T p g) f -> T p g f", p=128, g=G_SUPER)
            out_v = out.rearrange("(T p g) o -> T p g o", p=128, g=G_SUPER)

            for T in range(n_super):
                x_sm = xin.tile([128, G_SUPER, F_IN], f32, tag="xsm")
                nc.sync.dma_start(out=x_sm, in_=sig_v[T])

                for q in range(SUBQ):
                    st = T * SUBQ + q
                    # ---- transpose into feature-major (PSUM) ----
                    xA_ps = psA.tile([128, NSUB], f32, tag="p0")
                    xB_ps = psA.tile([19, NSUB], f32, tag="p1")
                    for j in range(4):
                        g = q * 4 + j
                        nc.tensor.transpose(
                            out=xA_ps[:, j * 128:(j + 1) * 128],
                            in_=x_sm[:, g, 0:128], identity=ident)
                        nc.tensor.transpose(
                            out=xB_ps[:, j * 128:(j + 1) * 128],
                            in_=x_sm[:, g, 128:F_USE], identity=ident)
                    xA = sbx.tile([128, NSUB], f32, tag="xA")
                    xB = sbx.tile([19, NSUB], f32, tag="xB")
                    nc.scalar.activation(out=xA, in_=xA_ps, func=AF.Copy)
                    nc.scalar.activation(out=xB, in_=xB_ps, func=AF.Copy)
                    x2A = sbx.tile([128, NSUB], f32, tag="x2A")
                    x2B = sbx.tile([19, NSUB], f32, tag="x2B")
                    nc.vector.tensor_mul(out=x2A, in0=xA, in1=xA)
                    nc.vector.tensor_mul(out=x2B, in0=xB, in1=xB)

                    # ---- windowed sums s' = A@x/sqrt(10), s2 = A@x^2 ----
                    s_ps = psA.tile([F_STD, NSUB], f32, tag="p2")
                    nc.tensor.matmul(s_ps, _w(weights, "sA_a"), xA,
                                     start=True, stop=False)
                    nc.tensor.matmul(s_ps, _w(weights, "sA_b"), xB,
                                     start=False, stop=True)
                    s2_ps = psA.tile([F_STD, NSUB], f32, tag="p3")
                    nc.tensor.matmul(s2_ps, _w(weights, "s2_a"), x2A,
                                     start=True, stop=False)
                    nc.tensor.matmul(s2_ps, _w(weights, "s2_b"), x2B,
                                     start=False, stop=True)

                    # ---- std = sqrt((s2 - s'^2)/9) ----
                    t_sb = sbx.tile([F_STD, NSUB], f32, tag="t")
                    nc.scalar.activation(out=t_sb, in_=s_ps, func=AF.Square)
                    u_sb = sbx.tile([F_STD, NSUB], f32, tag="u")
                    nc.vector.tensor_sub(out=u_sb, in0=s2_ps, in1=t_sb)
                    std = sbx.tile([F_STD, NSUB], f32, tag="std")
                    nc.scalar.activation(out=std, in_=u_sb, func=AF.Sqrt,
                                         scale=1.0 / 9.0)

                    # ---- conv1 (120 -> 190) + relu ----
                    h1a_ps = psA.tile([128, NSUB], f32, tag="p4")
                    nc.tensor.matmul(h1a_ps, _w(weights, "c1_a"), std)
                    h1b_ps = psA.tile([62, NSUB], f32, tag="p5")
                    nc.tensor.matmul(h1b_ps, _w(weights, "c1_b"), std)
                    h1a = sbh.tile([128, NSUB], f32, tag="h1a")
                    nc.scalar.activation(out=h1a, in_=h1a_ps, func=AF.Relu,
                                         bias=_w(weights, "b1a"))
                    h1b = sbh.tile([62, NSUB], f32, tag="h1b")
                    nc.scalar.activation(out=h1b, in_=h1b_ps, func=AF.Relu,
                                         bias=_w(weights, "b1b"))

                    # ---- conv2 (190 -> 360) + relu ----
                    h2 = []
                    mo_sz = (128, 128, 104)
                    for m in range(3):
                        tag = ("p6", "p7", "p2")[m]
                        h2_ps = psA.tile([mo_sz[m], NSUB], f32, tag=tag)
                        nc.tensor.matmul(h2_ps, _w(weights, f"c2a{m}"), h1a,
                                         start=True, stop=False)
                        nc.tensor.matmul(h2_ps, _w(weights, f"c2b{m}"), h1b,
                                         start=False, stop=True)
                        h2m = sbh.tile([mo_sz[m], NSUB], f32, tag=f"h2{m}")
                        nc.vector.tensor_scalar(
                            out=h2m, in0=h2_ps,
                            scalar1=_w(weights, f"b2_{m}"), scalar2=0.0,
                            op0=ALU.add, op1=ALU.max)
                        h2.append(h2m)

                    # ---- fc1 (360 -> 256) + relu ----
                    h3 = []
                    for m in range(2):
                        h3_ps = psA.tile([128, NSUB], f32, tag=("p4", "p5")[m])
                        for k in range(3):
                            nc.tensor.matmul(h3_ps, _w(weights, f"f1_{k}_{m}"),
                                             h2[k], start=(k == 0),
                                             stop=(k == 2))
                        h3m = sbh.tile([128, NSUB], f32, tag=f"h3{m}")
                        nc.vector.tensor_scalar(
                            out=h3m, in0=h3_ps,
                            scalar1=_w(weights, f"b3_{m}"), scalar2=0.0,
                            op0=ALU.add, op1=ALU.max)
                        h3.append(h3m)

                    # ---- fc2 (256 -> 64) + relu ----
                    h4_ps = psA.tile([F_FC2, NSUB], f32, tag="p6")
                    for k in range(2):
                        nc.tensor.matmul(h4_ps, _w(weights, f"f2_{k}"), h3[k],
                                         start=(k == 0), stop=(k == 1))
                    h4 = sbh.tile([F_FC2, NSUB], f32, tag="h4")
                    nc.vector.tensor_scalar(
                        out=h4, in0=h4_ps, scalar1=_w(weights, "b4"),
                        scalar2=0.0, op0=ALU.add, op1=ALU.max)

                    # ---- pamap via weight streaming: logits sample-major ----
                    lg_ps = psA.tile([128, 4 * F_OUT], f32, tag="p3")
                    for j in range(4):
                        nc.tensor.matmul(
                            lg_ps[:, j * F_OUT:(j + 1) * F_OUT],
                            h4[:, j * 128:(j + 1) * 128], _w(weights, "wp"),
                            start=True, stop=True)
                    nc.vector.tensor_copy(
                        out=logits_t[T][:, q * 48:(q + 1) * 48], in_=lg_ps)

            # ---------- phase B: log-softmax (exp/ln table set) ----------
            tc.no_sync_barrier()
            import concourse.bass as bass_mod
            w_bp = _w(weights, "bp")                       # [128, 12]
            bp3d = bass_mod.AP(tensor=w_bp.tensor, offset=w_bp.offset,
                               ap=[w_bp.ap[0], [0, G_SUPER], w_bp.ap[1]])
            for T in range(n_super):
                chunk = logits_t[T][:, :]
                ch3 = chunk.rearrange("p (g o) -> p g o", o=F_OUT)
                lb = outp.tile([128, G_SUPER, F_OUT], f32, tag="lb")
                nc.vector.tensor_tensor(out=lb, in0=ch3, in1=bp3d, op=ALU.add)
                e = outp.tile([128, G_SUPER, F_OUT], f32, tag="e")
                nc.scalar.activation(out=e, in_=lb, func=AF.Exp)
                ssum = outp.tile([128, G_SUPER], f32, tag="ss")
                nc.vector.tensor_reduce(out=ssum, in_=e,
                                        axis=mybir.AxisListType.X, op=ALU.add)
                lse = outp.tile([128, G_SUPER], f32, tag="lse")
                nc.scalar.activation(out=lse, in_=ssum, func=AF.Ln)
                lse3 = bass_mod.AP(tensor=lse.tensor, offset=lse.offset,
                                   ap=[lse.ap[0], lse.ap[1], [0, F_OUT]])
                ot = outp.tile([128, G_SUPER, F_OUT], f32, tag="ot")
                nc.vector.tensor_tensor(out=ot, in0=lb, in1=lse3,
                                        op=ALU.subtract)
                nc.sync.dma_start(out=out_v[T], in_=ot)

    _split_excess_waits(nc)
    return nc


def _get_program(n_super):
    global _PROGRAM
    if _PROGRAM is None or _PROGRAM[0] != n_super:
        _PROGRAM = (n_super, _build_program(n_super))
    return _PROGRAM[1]


# ---------------------------------------------------------------------------
# Entry point
# ---------------------------------------------------------------------------
def kernel(signal, conv1_w, conv1_b, conv2_w, conv2_b, fc1_w, fc1_b,
           fc2_w, fc2_b, pamap_w, pamap_b, **_unused):
    from concourse.bass_utils import run_bass_kernel_spmd

    n_super = _N_SUPER
    b_core = n_super * SUPER
    signal = np.asarray(signal, np.float32)
    b_tot = signal.shape[0]
    assert b_tot == b_core * NCORES, (b_tot, b_core)

    blob = _build_blob(np.asarray(conv1_w), np.asarray(conv1_b),
                       np.asarray(conv2_w), np.asarray(conv2_b),
                       np.asarray(fc1_w), np.asarray(fc1_b),
                       np.asarray(fc2_w), np.asarray(fc2_b),
                       np.asarray(pamap_w), np.asarray(pamap_b))

    nc = _get_program(n_super)
    sig_flat = np.ascontiguousarray(signal.reshape(b_tot, F_IN))
    in_maps = [{"sig": sig_flat[c * b_core:(c + 1) * b_core], "wb": blob}
               for c in range(NCORES)]
    res = run_bass_kernel_spmd(nc, in_maps, core_ids=list(range(NCORES)))
    outs = [res.results[c]["out"] for c in range(NCORES)]
    return np.concatenate(outs, axis=0)

